# revision 10
# baseline (speedup 1.0000x reference)
"""3x3 median blur on Trainium2, data-parallel across 8 NeuronCores.

Input:  image (16, 3, 512, 512) float32
Output: median-blur(3x3, zero-padded) same shape.

Strategy:
- Shard batch across 8 cores: core c handles images [2c, 2c+2) -> 6 channel
  planes of 512x512 each.
- fp16 end-to-end on device (monotone rounding commutes with median, so the
  only error is the final rounding of the true median: rel err <= 2^-11).
  fp16 also halves DMA bytes and doubles DVE throughput: tensor_tensor
  qualifies for the DVE 2x_1p perf mode (2 elems/cycle) only when every
  operand's innermost AP dim is unit-stride on a 2-byte dtype.
- Host pads each plane to 514x514 and DEINTERLEAVES columns per row:
  [E(257) | O(257)] with E[m] = col 2m, O[m] = col 2m+1. The 3-wide
  horizontal windows then read only unit-stride runs (the classic even/odd
  pair-sharing trick without stride-2 access), keeping every instruction in
  the 2x mode.
- Layout per pass (3 passes x 2 planes): partition p = 64h + c holds a
  10-row x 514-col deinterleaved slab (8 output rows + halo) of plane
  2t + h, rows 8c..8c+9.
- Median-of-9 as separable sorting network, ~15 min/max elems per output
  pixel, all packed fp16 on the vector engine:
    vertical sort3 (shared adjacent-row pairs) -> lo, mid, hi;
    horizontal: shared pair (O[m], E[m+1]) between outputs 2m and 2m+1:
      A = max3(lo), C = min3(hi), B = med3(mid); final med3(A, B, C).
- Output staged [pass][128][8*512] fp16, rows deinterleaved [E|O];
  host re-interleaves and casts back to fp32.
"""

import hashlib
import os
import shutil
import sys

if "/opt/trn_rl_repo" not in sys.path:
    sys.path.insert(0, "/opt/trn_rl_repo")

import numpy as np

import concourse.bass as bass
import concourse.tile as tile
from concourse import bacc, mybir
from concourse.bass_utils import run_bass_kernel_spmd

F16 = mybir.dt.float16
MAX = mybir.AluOpType.max
MIN = mybir.AluOpType.min

N_CORES = 8
B, C, H, W = 16, 3, 512, 512
PLANES = (B * C) // N_CORES  # 6 planes per core
PH, PW = H + 2, W + 2  # 514, 514
NE = PW // 2  # 257 even (and odd) columns per padded row

N_PASSES = PLANES // 2  # 2 planes per pass
CHUNK = 8  # output rows per partition per pass
SLAB = CHUNK + 2  # input rows per partition slab
SLABE = SLAB * PW  # slab elems per partition (5140)
OUTE = CHUNK * W  # output elems per partition per pass (4096)
NH = W // 2  # 256 outputs of each parity per row

_CACHED = {}

_NEFF_CACHE_DIR = "/tmp/bass_neff_cache"


def _install_neff_cache():
    """Memoise walrus compiles on disk, keyed by the BIR json hash."""
    if _CACHED.get("neff_cache"):
        return
    import concourse.bass2jax as b2j
    import concourse.bass_utils as bu

    orig = bu.compile_bir_kernel

    def cached_compile(bir_json, tmpdir, neff_name="file.neff"):
        key = hashlib.sha256(bir_json).hexdigest()
        cpath = os.path.join(_NEFF_CACHE_DIR, f"{key}.neff")
        dst = os.path.join(tmpdir, neff_name)
        if os.path.exists(cpath):
            shutil.copy(cpath, dst)
            return dst
        p = orig(bir_json, tmpdir, neff_name)
        try:
            os.makedirs(_NEFF_CACHE_DIR, exist_ok=True)
            tmp = cpath + ".tmp"
            shutil.copy(p, tmp)
            os.replace(tmp, cpath)
        except OSError:
            pass
        return p

    bu.compile_bir_kernel = cached_compile
    b2j.compile_bir_kernel = cached_compile
    _CACHED["neff_cache"] = True


def _ap(apref, off, dims):
    """View into a tile AP with explicit [step, num] free dims."""
    part = list(apref.ap[0])
    return bass.AP(apref.tensor, apref.offset + off, [part] + [list(d) for d in dims])


def _dram(handle, off, dims):
    return bass.AP(handle, off, [list(d) for d in dims])


def _build():
    nc = bacc.Bacc(
        "TRN2", target_bir_lowering=False, debug=False, num_devices=N_CORES
    )
    xin = nc.dram_tensor("xs", [N_PASSES, 128, SLABE], F16, kind="ExternalInput")
    yout = nc.dram_tensor("ys", [N_PASSES, 128, OUTE], F16, kind="ExternalOutput")

    with tile.TileContext(nc) as tc:
        _body(tc, nc, xin, yout)

    nc.compile()
    return nc


def _body(tc, nc, xin, yout):
    from contextlib import ExitStack

    ctx = ExitStack()
    with ctx:
        xpool = ctx.enter_context(tc.tile_pool(name="xpool", bufs=2))
        vpool = ctx.enter_context(tc.tile_pool(name="vpool", bufs=1))
        lmh = ctx.enter_context(tc.tile_pool(name="lmh", bufs=1))
        hpool = ctx.enter_context(tc.tile_pool(name="hpool", bufs=1))
        abc = ctx.enter_context(tc.tile_pool(name="abc", bufs=1))
        fin = ctx.enter_context(tc.tile_pool(name="fin", bufs=1))
        opool = ctx.enter_context(tc.tile_pool(name="opool", bufs=2))

        tt = nc.vector.tensor_tensor

        for t in range(N_PASSES):
            X = xpool.tile([128, SLABE], F16, name="X")
            if t == 0:
                # Split the first input DMA so compute starts as soon as
                # slab rows 0..5 land (1-elem WAW overlap orders chunk B
                # after chunk A, so A runs at full rate).
                ca = 6 * PW + 1
                for e0, e1 in ((0, ca), (ca - 1, SLABE)):
                    nc.sync.dma_start(
                        X[:, e0:e1],
                        _dram(
                            xin, e0, [[SLABE, 128], [1, e1 - e0]]
                        ),
                    )
            else:
                nc.sync.dma_start(
                    X[:, :],
                    _dram(xin, t * 128 * SLABE, [[SLABE, 128], [1, SLABE]]),
                )

            # ---- vertical: column sort3 -> lo, mid, hi ----
            # pairs k=0..3 at slab rows (2k+1, 2k+2); triple for output row
            # r = 2k+pol uses pair k and third row 2k + 3*pol.
            pmin = vpool.tile([128, 4 * PW], F16, name="pmin")
            pmax = vpool.tile([128, 4 * PW], F16, name="pmax")
            # lo/mid/hi packed in ONE tile so horizontal-stage instructions
            # can pair same-ALU-op work via a leading select dim.
            CP = CHUNK * PW
            lmh3 = lmh.tile([128, 3 * CP], F16, name="lmh3")
            LO, MID, HI = 0, CP, 2 * CP
            u = vpool.tile([128, CHUNK * PW], F16, name="u")

            # On pass 0, emit the vertical stage in two 4-row halves so the
            # first half only depends on DMA chunk A (rows 0..5).
            for h in range(2) if t == 0 else (0,):
                nk = 2 if t == 0 else 4  # pairs per emitted instruction
                kb = 2 * h  # first pair index
                pv = [[PW, nk], [1, PW]]
                pr = [[2 * PW, nk], [1, PW]]
                po = kb * PW
                xo = 2 * kb * PW
                tt(_ap(pmin, po, pv), _ap(X, xo + PW, pr), _ap(X, xo + 2 * PW, pr), MIN)
                tt(_ap(pmax, po, pv), _ap(X, xo + PW, pr), _ap(X, xo + 2 * PW, pr), MAX)

                vout = [[2 * PW, nk], [PW, 2], [1, PW]]  # row 2k+pol
                vbcast = [[PW, nk], [0, 2], [1, PW]]  # pair k, pol-broadcast
                third = _ap(X, xo, [[2 * PW, nk], [3 * PW, 2], [1, PW]])
                pm = _ap(pmin, po, vbcast)
                pM = _ap(pmax, po, vbcast)
                tt(_ap(lmh3, LO + xo, vout), pm, third, MIN)
                tt(_ap(lmh3, HI + xo, vout), pM, third, MAX)
                tt(_ap(u, xo, vout), pM, third, MIN)
                tt(_ap(lmh3, MID + xo, vout), pm, _ap(u, xo, vout), MAX)

            # ---- horizontal, deinterleaved ----
            # per padded row [E(257) | O(257)]: output 2m reads E[m],O[m],
            # E[m+1]; output 2m+1 reads O[m],E[m+1],O[m+1]. Shared pair
            # (O[m], E[m+1]) = offsets (NE+m, 1+m). All unit-stride.
            # Same-ALU-op instruction pairs are fused via a leading select
            # dim over co-located tiles: {s,x} = MAX over (lo,mid) shifted
            # views; {t,n} = MIN over (hi,mid).
            CN = CHUNK * NH
            stx = hpool.tile([128, 2 * CN], F16, name="stx")  # s@0, x@CN
            tn = hpool.tile([128, 2 * CN], F16, name="tn")  # t@0, n@CN
            uv = hpool.tile([128, CHUNK * W], F16, name="uv")
            AM = abc.tile([128, 2 * OUTE], F16, name="AM")  # A@0, mx@OUTE
            BC = abc.tile([128, 2 * OUTE], F16, name="BC")  # B@0, C@OUTE

            pair2 = [[CN, 2], [NH, CHUNK], [1, NH]]

            def lmh2(base, off, sel):  # two lmh3 planes, shifted by off
                return _ap(lmh3, base + off, [[sel, 2], [PW, CHUNK], [1, NH]])

            tt(_ap(stx, 0, pair2), lmh2(LO, NE, CP), lmh2(LO, 1, CP), MAX)
            tt(_ap(tn, 0, pair2), lmh2(HI, NE, -CP), lmh2(HI, 1, -CP), MIN)

            # fused even/odd combine: out row layout [E(256) | O(256)];
            # the eo dim selects third-col offset 0 (even) / NE+1 (odd)
            # while the pair term broadcasts across eo.
            oeo = [[W, CHUNK], [NH, 2], [1, NH]]
            beo = [[NH, CHUNK], [0, 2], [1, NH]]

            def teo(base):  # thirds: E[m] (even) then O[m+1] at NE+1 (odd)
                return _ap(lmh3, base, [[PW, CHUNK], [NE + 1, 2], [1, NH]])

            tt(_ap(AM, 0, oeo), _ap(stx, 0, beo), teo(LO), MAX)
            tt(_ap(BC, OUTE, oeo), _ap(tn, 0, beo), teo(HI), MIN)
            tt(_ap(uv, 0, oeo), _ap(stx, CN, beo), teo(MID), MIN)
            tt(_ap(BC, 0, oeo), _ap(tn, CN, beo), _ap(uv, 0, oeo), MAX)

            # ---- final med3(A, B, C) ----
            #   mx = max(A,B); {mn,t2} = min({A,mx},{B,C}); res = max(mn,t2)
            # On the last pass, emit in two halves so the first half's
            # output DMA overlaps the second half's compute (shorter tail).
            MT = fin.tile([128, 2 * OUTE], F16, name="MT")  # mn@0, t2@OUTE
            res = opool.tile([128, OUTE], F16, name="res")
            # Uneven split: chunk-1's DMA (~2.3us) hides under chunk-2's
            # compute (~2.6us), leaving only the small chunk-2 DMA exposed.
            halves = (
                ((0, 3 * OUTE // 4), (3 * OUTE // 4, OUTE))
                if t == N_PASSES - 1
                else ((0, OUTE),)
            )
            for f0, f1 in halves:
                fl = [[1, f1 - f0]]
                fl2 = [[OUTE, 2], [1, f1 - f0]]
                tt(_ap(AM, OUTE + f0, fl), _ap(AM, f0, fl), _ap(BC, f0, fl), MAX)
                tt(_ap(MT, f0, fl2), _ap(AM, f0, fl2), _ap(BC, f0, fl2), MIN)
                tt(_ap(res, f0, fl), _ap(MT, f0, fl), _ap(MT, OUTE + f0, fl), MAX)
                nc.sync.dma_start(
                    _dram(
                        yout,
                        t * 128 * OUTE + f0,
                        [[OUTE, 128], [1, f1 - f0]],
                    ),
                    res[:, f0:f1],
                )


def _get_nc():
    if "nc" not in _CACHED:
        _install_neff_cache()
        _CACHED["nc"] = _build()
    return _CACHED["nc"]


# staged-input row gather: for each chunk c (0..63), padded rows 8c..8c+10
_ROWIDX = (np.arange(64) * CHUNK)[:, None] + np.arange(SLAB)[None, :]


def _stage_input(shard6: np.ndarray) -> np.ndarray:
    """(6, 512, 512) fp32 -> [3, 128, SLABE] fp16 deinterleaved slabs."""
    padded = np.zeros((PLANES, PH, PW), dtype=np.float16)
    padded[:, 1:-1, 1:-1] = shard6
    # deinterleave columns: [E(257) | O(257)]
    deint = np.empty_like(padded)
    deint[:, :, :NE] = padded[:, :, 0::2]
    deint[:, :, NE:] = padded[:, :, 1::2]
    slabs = deint[:, _ROWIDX, :]  # (6, 64, 10, 514)
    return slabs.reshape(N_PASSES, 128, SLABE)


def _unstage_output(ys: np.ndarray) -> np.ndarray:
    """[3, 128, OUTE] fp16 -> (6, 512, 512) fp32 (re-interleave columns)."""
    # ys[t, 64h + c, r*512 + eo*256 + m] = plane(2t+h)[8c+r, 2m+eo]
    arr = ys.reshape(N_PASSES, 2, 64, CHUNK, 2, NH)  # (t, h, c, r, eo, m)
    arr = arr.transpose(0, 1, 2, 3, 5, 4)  # (t, h, c, r, m, eo)
    return arr.reshape(PLANES, H, W).astype(np.float32)


def kernel(image: np.ndarray, _trace: bool = False):
    assert image.shape == (B, C, H, W) and image.dtype == np.float32
    nc = _get_nc()

    per_core = B // N_CORES
    in_maps = []
    for c in range(N_CORES):
        shard = image[c * per_core : (c + 1) * per_core].reshape(PLANES, H, W)
        in_maps.append({"xs": _stage_input(shard)})

    res = run_bass_kernel_spmd(nc, in_maps, list(range(N_CORES)), trace=_trace)
    _CACHED["last_exec_ns"] = res.exec_time_ns

    out = np.empty((B, C, H, W), dtype=np.float32)
    for c in range(N_CORES):
        out[c * per_core : (c + 1) * per_core] = _unstage_output(
            res.results[c]["ys"]
        ).reshape(per_core, C, H, W)
    return out
